# revision 1
# baseline (speedup 1.0000x reference)
"""Multi-head attention (axis-swapped variant) on 8 Trainium2 NeuronCores.

Reference semantics (EMB=1024, heads act on the d_head axis after the buggy
transpose): 64 effective heads of size 16, causal softmax scaled by
1/sqrt(16), projections Wq/Wk/Wv, output projection Wo + bo.

Sharding: core c = 4*b + g handles batch b and head-group g (16 heads =
256 contiguous projection columns). Each core returns a partial output
[1024, 1024]; the host sums the 4 group partials per batch and adds bo.
"""

import numpy as np

import concourse.bass as bass
import concourse.mybir as mybir
import concourse.tile as tile
from concourse.bass_utils import run_bass_kernel_spmd

F32 = mybir.dt.float32
F32R = mybir.dt.float32r
BF16 = mybir.dt.bfloat16

EMB = 1024
SEQ = 1024
BATCH = 2
NG = 4            # head groups (cores per batch)
HPG = 16          # heads per group/core
DH = 16           # per-head feature size
GCOLS = HPG * DH  # 256 projection columns per core


def split_excess_waits(nc, cap=1):
    """This container's walrus rejects instructions carrying more than a few
    semaphore waits (and bass's own model says one). Relocate excess waits
    onto preceding same-engine EventSemaphore instructions."""

    def fix_block(bb, dummy):
        insts = bb.instructions
        i = 0
        while i < len(insts):
            inst = insts[i]
            si = inst.sync_info
            waits = list(si.on_wait) if si is not None and si.on_wait else []
            if len(waits) > cap:
                eng = nc.engines[inst.engine]
                excess, keep = waits[:-cap], waits[-cap:]
                si.on_wait = keep
                pos = i
                for j in range(0, len(excess), cap):
                    chunk = excess[j : j + cap]
                    ev = eng.wait_ge(dummy, 1)
                    cur_list = nc.cur_bb.bb.instructions
                    assert cur_list[-1] is ev.ins
                    cur_list.pop()
                    ev.ins.sync_info.on_wait = chunk
                    insts.insert(pos, ev.ins)
                    pos += 1
                    i += 1
            i += 1

    with nc.semaphore("waitfix_dummy") as dummy:
        for f in nc.m.functions:
            for bb in f.blocks:
                fix_block(bb, dummy)


def _r(ap):
    return ap.bitcast(F32R)


def build_nc():
    nc = bass.Bass()
    xT_d = nc.declare_dram_parameter("xT", [EMB, SEQ], F32R, isOutput=False)
    wq_d = nc.declare_dram_parameter("wq", [EMB, GCOLS], F32R, isOutput=False)
    wk_d = nc.declare_dram_parameter("wk", [EMB, GCOLS], F32R, isOutput=False)
    wv_d = nc.declare_dram_parameter("wv", [EMB, GCOLS], F32R, isOutput=False)
    wo_d = nc.declare_dram_parameter("wo", [512, EMB], F32R, isOutput=False)
    y_d = nc.declare_dram_parameter("y", [SEQ, EMB], F32, isOutput=True)


    with tile.TileContext(nc) as tc:
        with (
            tc.tile_pool(name="big", bufs=1) as big,
            tc.tile_pool(name="work", bufs=3) as work,
            tc.tile_pool(name="att", bufs=6) as att,
            tc.tile_pool(name="dram", bufs=1, space="DRAM") as dram,
        ):
            drc = dram.tile([HPG, SEQ], F32)   # gathered denominators
            drr = dram.tile([HPG, SEQ], F32)   # their reciprocals
            # ---- stage A: load everything (weights first, xT chunked
            # so the first projection matmuls start early) ----
            XT = big.tile([128, 8, SEQ], F32R)      # [d_in_block, kb, m]
            xT_r = xT_d[:].rearrange("(kb p) m -> p kb m", p=128)
            WQ = big.tile([128, 8, GCOLS], F32R)
            nc.sync.dma_start(WQ[:], wq_d[:].rearrange("(kb p) n -> p kb n", p=128))
            for kb in range(8):
                nc.sync.dma_start(XT[:, kb, :], xT_r[:, kb, :])
            WK = big.tile([128, 8, GCOLS], F32R)
            nc.sync.dma_start(WK[:], wk_d[:].rearrange("(kb p) n -> p kb n", p=128))
            WV = big.tile([128, 8, GCOLS], F32R)
            nc.sync.dma_start(WV[:], wv_d[:].rearrange("(kb p) n -> p kb n", p=128))
            WO = big.tile([128, 4, EMB], F32R)
            nc.sync.dma_start(WO[:], wo_d[:].rearrange("(t p) n -> p t n", p=128))

            # v-augmented weights: per (jb, head) a [128, 32] block =
            # [v(16 cols) | 1 | zeros(15)]
            VA = big.tile([128, 8, HPG, 32], BF16)
            nc.gpsimd.memset(VA[:], 0.0)
            nc.gpsimd.memset(VA[:, :, :, 16:17], 1.0)

            # ---- stage B: projections (fp32r) ----
            # qT/kT: out[col, m] = sum_d W[d, col] * xT[d, m]
            QT = big.tile([128, 2, SEQ], BF16)
            KT = big.tile([128, 2, SEQ], BF16)
            ps_proj_cm = tc.tile_pool(name="ps_proj", bufs=2, space="PSUM")
            ps_proj = ps_proj_cm.__enter__()
            for W, T in ((WQ, QT), (WK, KT)):
                for t in range(2):
                    for ic in range(2):
                        pq = ps_proj.tile([128, 512], F32, tag="pproj")
                        for kb in range(8):
                            nc.tensor.matmul(
                                pq[:],
                                W[:, kb, 128 * t : 128 * t + 128],
                                XT[:, kb, 512 * ic : 512 * ic + 512],
                                start=(kb == 0),
                                stop=(kb == 7),
                            )
                        nc.vector.tensor_copy(
                            T[:, t, 512 * ic : 512 * ic + 512], pq[:]
                        )
            # v natural: out[m, col] = sum_d xT[d, m] * W[d, col]; write
            # straight into VA's per-head v columns (bf16 cast on copy).
            for mt in range(8):
                pv = ps_proj.tile([128, GCOLS], F32, tag="pproj")
                for kb in range(8):
                    nc.tensor.matmul(
                        pv[:],
                        XT[:, kb, 128 * mt : 128 * mt + 128],
                        WV[:, kb, :],
                        start=(kb == 0),
                        stop=(kb == 7),
                    )
                nc.vector.tensor_copy(
                    VA[:, mt, :, 0:16],
                    pv[:].rearrange("p (h e) -> p h e", e=DH),
                )

            ps_proj_cm.__exit__(None, None, None)

            # shifted copies: odd heads moved down 16 partitions so every
            # head's 16 rows start at a 32-aligned partition
            QTs = big.tile([128, 2, SEQ], BF16)
            KTs = big.tile([128, 2, SEQ], BF16)
            for src, dst in ((QT, QTs), (KT, KTs)):
                for j in range(4):
                    nc.sync.dma_start(
                        dst[32 * j : 32 * j + 16, :, :],
                        src[32 * j + 16 : 32 * j + 32, :, :],
                    )

            def head_slice(T, Ts, h, lo, size):
                t, hl = divmod(h, 8)
                src = T if hl % 2 == 0 else Ts
                base = 16 * (hl - hl % 2)
                return src[base : base + 16, t, lo : lo + size]

            # ---- stage C: attention ----
            # i-chunk-outer so ctx PSUM is 1 bank; scores tiles 2 banks x3
            # bufs so PE/ACT/Pool pipeline across (pair, jb)
            SCr = []
            SC = []
            for t2 in range(4):
                SCr.append(big.tile([128, SEQ], F32, tag=f"scr{t2}", name=f"scr{t2}"))
            with (
                tc.tile_pool(name="ps_sw", bufs=3, space="PSUM") as ps_sw,
                tc.tile_pool(name="ps_cp", bufs=2, space="PSUM") as ps_cp,
            ):
                for t2 in range(4):
                    # quad heads {0,2,4,6}+off: one source tile, bases
                    # {0,32,64,96} -> 4 concurrent row-groups on the PE
                    toff = (t2 // 2) * 8
                    quad = [toff + 2 * q + (t2 % 2) for q in range(4)]
                    for ic in range(2):
                        c0 = 512 * ic
                        CP = ps_cp.tile([128, 512], F32, tag="cp")
                        bank_first = [True] * 4
                        for jb in range(2 * (ic + 1) * 2):
                            i0 = 128 * jb
                            lo = max(c0, i0)
                            hi = c0 + 512
                            if lo >= hi:
                                continue
                            SWs, ATs = [], []
                            for pr in range(2):
                                SW = ps_sw.tile([128, 2, 512], F32, tag="sw")
                                AT = att.tile([128, 2, 512], BF16, tag="at")
                                SWs.append(SW)
                                ATs.append(AT)
                                for ph in range(2):
                                    h = quad[2 * pr + ph]
                                    hl = h % 8
                                    tp = (
                                        (96, 0)
                                        if 16 * (hl - hl % 2) == 96
                                        else None
                                    )
                                    nc.tensor.matmul(
                                        SW[:, ph, lo - c0 : hi - c0],
                                        head_slice(KT, KTs, h, i0, 128),
                                        head_slice(QT, QTs, h, lo, hi - lo),
                                        start=True,
                                        stop=True,
                                        tile_position=tp,
                                    )
                            for pr in range(2):
                                nc.scalar.activation(
                                    ATs[pr][:, :, lo - c0 : 512],
                                    SWs[pr][:, :, lo - c0 : 512],
                                    mybir.ActivationFunctionType.Exp,
                                    scale=0.25,
                                )
                            if lo == i0:
                                for pr in range(2):
                                    # diagonal block: keep j <= i
                                    nc.gpsimd.affine_select(
                                        out=ATs[pr][:, :, lo - c0 : lo - c0 + 128],
                                        in_=ATs[pr][:, :, lo - c0 : lo - c0 + 128],
                                        compare_op=mybir.AluOpType.is_ge,
                                        fill=0.0,
                                        base=0,
                                        pattern=[[0, 2], [1, 128]],
                                        channel_multiplier=-1,
                                    )
                            for pr in range(2):
                                for ph in range(2):
                                    h = quad[2 * pr + ph]
                                    cg = 2 * pr + ph
                                    nc.tensor.matmul(
                                        CP[32 * cg : 32 * cg + 32, lo - c0 : 512],
                                        VA[:, jb, h, :],
                                        ATs[pr][:, ph, lo - c0 : 512],
                                        start=bank_first[cg],
                                        stop=False,
                                        tile_position=(0, 32 * cg),
                                        skip_group_check=True,
                                    )
                                    bank_first[cg] = False
                        # evacuate this i-chunk of ctxT_aug
                        nc.vector.tensor_copy(
                            SCr[t2][:, c0 : c0 + 512], CP[:]
                        )
                    # gather the 4 denominator rows {16,48,80,112} -> DRAM
                    s = SCr[t2][:]
                    nc.sync.dma_start(
                        drc[4 * t2 : 4 * t2 + 4, :],
                        bass.AP(
                            tensor=s.tensor,
                            offset=s[16:17].offset,
                            ap=[[32 * s.ap[0][0], 4]] + s[16:17].ap[1:],
                        ),
                    )
                    # per-quad reciprocal + broadcast + divide, overlapping
                    # the next quad's attention
                    DSQ = work.tile([32, 128], F32, tag="dsq")
                    nc.sync.dma_start(
                        DSQ[:],
                        drc[4 * t2 : 4 * t2 + 4, :].rearrange(
                            "h (pi f) -> (h pi) f", pi=8
                        ),
                    )
                    RSQ = work.tile([32, 128], F32, tag="rsq")
                    nc.vector.reciprocal(out=RSQ[:], in_=DSQ[:])
                    nc.sync.dma_start(
                        drr[4 * t2 : 4 * t2 + 4, :].rearrange(
                            "h (pi f) -> (h pi) f", pi=8
                        ),
                        RSQ[:],
                    )
                    B = big.tile([128, SEQ], F32, tag=f"bc{t2}", name=f"bc{t2}")
                    for c2 in range(4):
                        h = 4 * t2 + c2
                        nc.sync.dma_start(
                            B[32 * c2 : 32 * c2 + 32, :],
                            drr[h : h + 1, :].to_broadcast([32, SEQ]),
                        )
                    Sd = big.tile([128, SEQ], F32R, tag=f"sc{t2}", name=f"sc{t2}")
                    nc.vector.tensor_mul(Sd[:], SCr[t2][:], B[:])
                    SC.append(Sd)


            # ---- stage D: output projection (fp32r) ----
            with tc.tile_pool(name="ps_o", bufs=2, space="PSUM") as ps_o:
                for ib in range(8):
                    po = ps_o.tile([128, EMB], F32, tag="po")
                    for ic in range(2):
                        for t2 in range(4):
                            nc.tensor.matmul(
                                po[:, 512 * ic : 512 * ic + 512],
                                SC[t2][:, 128 * ib : 128 * ib + 128],
                                WO[:, t2, 512 * ic : 512 * ic + 512],
                                start=(t2 == 0),
                                stop=(t2 == 3),
                            )
                    Y = work.tile([128, EMB], F32, tag="y")
                    nc.vector.tensor_copy(Y[:], po[:])
                    nc.sync.dma_start(y_d[128 * ib : 128 * ib + 128, :], Y[:])

    split_excess_waits(nc)
    return nc


_NC_CACHE = None


def _get_nc():
    global _NC_CACHE
    if _NC_CACHE is None:
        _NC_CACHE = build_nc()
    return _NC_CACHE


def kernel(x, Wq, Wk, Wv, Wo, bo):
    x = np.asarray(x, dtype=np.float32)
    Wq = np.asarray(Wq, dtype=np.float32)
    Wk = np.asarray(Wk, dtype=np.float32)
    Wv = np.asarray(Wv, dtype=np.float32)
    Wo = np.asarray(Wo, dtype=np.float32)
    bo = np.asarray(bo, dtype=np.float32)

    nc = _get_nc()
    in_maps = []
    for c in range(8):
        b, g = divmod(c, NG)
        cols = slice(GCOLS * g, GCOLS * g + GCOLS)
        # Wo rows for this group, padded to the ctx-psum row layout:
        # quad t2, col-group c2, row r<16 -> Wo[g*256 + (4*t2+c2)*16 + r]
        wo_aug = np.zeros((512, EMB), dtype=np.float32)
        wo_g = Wo[cols, :]
        for t2 in range(4):
            for c2 in range(4):
                h = (t2 // 2) * 8 + 2 * c2 + (t2 % 2)
                wo_aug[128 * t2 + 32 * c2 : 128 * t2 + 32 * c2 + 16, :] = wo_g[
                    16 * h : 16 * h + 16, :
                ]
        in_maps.append(
            {
                "xT": np.ascontiguousarray(x[b].T),
                "wq": np.ascontiguousarray(Wq[:, cols]),
                "wk": np.ascontiguousarray(Wk[:, cols]),
                "wv": np.ascontiguousarray(Wv[:, cols]),
                "wo": wo_aug,
            }
        )

    res = run_bass_kernel_spmd(nc, in_maps, core_ids=list(range(8)))
    out = np.zeros((BATCH, SEQ, EMB), dtype=np.float32)
    for c in range(8):
        b = c // NG
        out[b] += res.results[c]["y"]
    out += bo[None, None, :]
    return out



# revision 34
# speedup vs baseline: 1.2454x; 1.2454x over previous
"""Multi-head attention (axis-swapped variant) on 8 Trainium2 NeuronCores.

Reference semantics (EMB=1024, heads act on the d_head axis after the buggy
transpose): 64 effective heads of size 16, causal softmax scaled by
1/sqrt(16), projections Wq/Wk/Wv, output projection Wo + bo.

Sharding: core c = 4*b + g handles batch b and head-group g (16 heads =
256 contiguous projection columns). Each core returns a partial output
[1024, 1024] (bf16); the host sums the 4 group partials per batch and
adds bo.

Per-core structure:
  B) Q/K/V projections in bf16 (weights and x pre-cast on host).
  C) per (query-block qb, key-block kb<=qb) step:
     - 16 scores matmuls [128 keys, 128 queries] (one per head) into two
       2-bank PSUM tiles. Hardware constraint: all matmuls into one PSUM
       bank must share the stationary row-base (tile_position row), so
       heads are grouped per bank by their 32-aligned partition base:
       tile0 = heads {0,1,8,9 | 2,3,10,11}, tile1 = {4,5,12,13 |
       6,7,14,15}.
     - exp: Act engine takes tile0 + 1 slot (native Exp, scale 1/4), DVE
       takes 7 slots via one-op Schraudolph (int16(s*C+B) bitcast bf16,
       ~3% rel err on those heads).
     - diagonal steps: one GPSIMD affine_select zeroes the j>i half of
       the exp'd tile (causal mask), instead of masking scores.
     - ctx accumulated transposed: stationary = attn tile [128k, 128q],
       moving = V_aug [128k, 17] (16 v cols + ones col). Output
       [128 q, 16 heads * 17] accumulates over kb in one PSUM bank; the
       ones column yields the softmax denominator per (q, head). ctx is
       emitted 2 steps late so PE never waits on exp/affsel.
     - after the diagonal step: reciprocal of the 16 denominator cols,
       broadcast-multiply normalizes -> CTXD[qb] [128 q, 256] bf16.
  D) per qb: PE-transpose CTXD into [256 he, 128 q] (bf16 identity),
     out-projection po[q, d] = sum_he CT[he, q] * Wo[he, d], store y.
"""

import numpy as np
import ml_dtypes

import concourse.bass as bass
import concourse.mybir as mybir
import concourse.tile as tile
from concourse.bass_utils import run_bass_kernel_spmd

F32 = mybir.dt.float32
BF16 = mybir.dt.bfloat16
I16 = mybir.dt.int16

EMB = 1024
SEQ = 1024
BATCH = 2
NG = 4            # head groups (cores per batch)
HPG = 16          # heads per group/core
DH = 16           # per-head feature size
GCOLS = HPG * DH  # 256 projection columns per core

NQB = 8           # query blocks of 128
CTX_DELAY = 2     # steps between scores emission and ctx consumption

SCH_C = 128.0 * float(np.log2(np.e)) * 0.25
SCH_B = 127.0 * 128.0 - 7.0


def head_of(ti, slot):
    """Head stored in scores tile `ti` slot `slot` (4 slots per bank; each
    bank holds only heads whose 16-row q/k slice starts at one 32-aligned
    partition base)."""
    bi = 2 * ti + slot // 4
    i = slot % 4
    return [2 * bi, 2 * bi + 1, 8 + 2 * bi, 8 + 2 * bi + 1][i]


def split_excess_waits(nc, cap=1):
    """This container's walrus rejects instructions carrying more than a few
    semaphore waits (and bass's own model says one). Relocate excess waits
    onto preceding same-engine EventSemaphore instructions."""

    def fix_block(bb, dummy):
        insts = bb.instructions
        i = 0
        while i < len(insts):
            inst = insts[i]
            si = inst.sync_info
            waits = list(si.on_wait) if si is not None and si.on_wait else []
            if len(waits) > cap:
                eng = nc.engines[inst.engine]
                excess, keep = waits[:-cap], waits[-cap:]
                si.on_wait = keep
                pos = i
                for j in range(0, len(excess), cap):
                    chunk = excess[j : j + cap]
                    ev = eng.wait_ge(dummy, 1)
                    cur_list = nc.cur_bb.bb.instructions
                    assert cur_list[-1] is ev.ins
                    cur_list.pop()
                    ev.ins.sync_info.on_wait = chunk
                    insts.insert(pos, ev.ins)
                    pos += 1
                    i += 1
            i += 1

    with nc.semaphore("waitfix_dummy") as dummy:
        for f in nc.m.functions:
            for bb in f.blocks:
                fix_block(bb, dummy)


def build_nc(debug=False):
    nc = bass.Bass()
    xt_d = nc.declare_dram_parameter("xt", [128, 8, SEQ], BF16, isOutput=False)
    wq_d = nc.declare_dram_parameter("wq", [128, 8, GCOLS], BF16, isOutput=False)
    wk_d = nc.declare_dram_parameter("wk", [128, 8, GCOLS], BF16, isOutput=False)
    wv_d = nc.declare_dram_parameter("wv", [128, 8, GCOLS], BF16, isOutput=False)
    wo_d = nc.declare_dram_parameter("wo", [128, 2, EMB], BF16, isOutput=False)
    id_d = nc.declare_dram_parameter("ident", [128, 128], BF16, isOutput=False)
    y_d = nc.declare_dram_parameter("y", [SEQ, EMB], BF16, isOutput=True)
    if debug:
        dbg_at = nc.declare_dram_parameter("dbg_at", [128, HPG, 128], BF16, isOutput=True)
        dbg_cx = nc.declare_dram_parameter("dbg_cx", [128, HPG, DH], BF16, isOutput=True)

    with tile.TileContext(nc) as tc:
        with (
            tc.tile_pool(name="big", bufs=1) as big,
            tc.tile_pool(name="work", bufs=4) as work,
        ):
            # ---- stage A: loads (wq first so projections start early) ----
            XT = big.tile([128, 8, SEQ], BF16)
            WQ = big.tile([128, 8, GCOLS], BF16)
            WK = big.tile([128, 8, GCOLS], BF16)
            WV = big.tile([128, 8, GCOLS], BF16)
            WO = big.tile([128, 2, EMB], BF16)
            IDENT = big.tile([128, 128], BF16)
            for kb in range(8):
                nc.sync.dma_start(WQ[:, kb, :], wq_d[:, kb, :])
                nc.sync.dma_start(XT[:, kb, :], xt_d[:, kb, :])
            for kb in range(8):
                nc.sync.dma_start(WK[:, kb, :], wk_d[:, kb, :])
            for kb in range(8):
                nc.sync.dma_start(WV[:, kb, :], wv_d[:, kb, :])
            nc.sync.dma_start(IDENT[:], id_d[:])
            for cb in range(2):
                nc.sync.dma_start(WO[:, cb, :], wo_d[:, cb, :])

            # v-augmented moving tiles: per (kb, head) a [128, 17] block =
            # [v(16 cols) | ones]
            VA = big.tile([128, 8, HPG, 17], BF16)
            nc.gpsimd.memset(VA[:, :, :, 16:17], 1.0)

            # ---- stage B: projections (bf16) ----
            QT = big.tile([128, 2, SEQ], BF16)
            KT = big.tile([128, 2, SEQ], BF16)
            with tc.tile_pool(name="ps_proj", bufs=2, space="PSUM") as ps_proj:
                for W, T in ((WQ, QT), (WK, KT)):
                    for t in range(2):
                        for ic in range(2):
                            pq = ps_proj.tile([128, 512], F32, tag="pproj")
                            for kb in range(8):
                                nc.tensor.matmul(
                                    pq[:],
                                    W[:, kb, 128 * t : 128 * t + 128],
                                    XT[:, kb, 512 * ic : 512 * ic + 512],
                                    start=(kb == 0),
                                    stop=(kb == 7),
                                )
                            nc.scalar.copy(
                                T[:, t, 512 * ic : 512 * ic + 512], pq[:]
                            )
                # v natural: out[m, col] = sum_d xT[d, m] * W[d, col]
                for mt in range(8):
                    pv = ps_proj.tile([128, GCOLS], F32, tag="pproj")
                    for kb in range(8):
                        nc.tensor.matmul(
                            pv[:],
                            XT[:, kb, 128 * mt : 128 * mt + 128],
                            WV[:, kb, :],
                            start=(kb == 0),
                            stop=(kb == 7),
                        )
                    nc.vector.tensor_copy(
                        VA[:, mt, :, 0:16],
                        pv[:].rearrange("p (h e) -> p h e", e=DH),
                    )

            # shifted copies: odd heads moved down 16 partitions so every
            # head's 16 rows start at a 32-aligned partition
            QTs = big.tile([128, 2, SEQ], BF16)
            KTs = big.tile([128, 2, SEQ], BF16)
            for src, dst in ((QT, QTs), (KT, KTs)):
                for j in range(4):
                    nc.sync.dma_start(
                        dst[32 * j : 32 * j + 16, :, :],
                        src[32 * j + 16 : 32 * j + 32, :, :],
                    )

            def head_slice(T, Ts, h, blk):
                t, hl = divmod(h, 8)
                src = T if hl % 2 == 0 else Ts
                base = 16 * (hl - hl % 2)
                return src[base : base + 16, t, 128 * blk : 128 * blk + 128]

            # ---- stage C: attention ----
            CTXD = [
                big.tile([128, HPG, DH], BF16, tag=f"ctxd{qb}", name=f"ctxd{qb}")
                for qb in range(NQB)
            ]
            steps = [(qb, kb) for qb in range(NQB) for kb in range(qb + 1)]
            with (
                tc.tile_pool(name="ps_sc", bufs=3, space="PSUM") as ps_sc,
                tc.tile_pool(name="ps_ctx", bufs=2, space="PSUM") as ps_ctx,
            ):
                ctx_ps = {}
                pending = []

                def emit_ctx(qb, kb, AT):
                    # start=True resets the whole PSUM bank (pending-zero
                    # covers the bank), so only the first matmul into the
                    # bank carries it; first writes to untouched regions
                    # still initialize via the per-byte pending-zero flags.
                    cp = ctx_ps[qb]
                    for slot in range(HPG):
                        h = head_of(slot // 8, slot % 8)
                        nc.tensor.matmul(
                            cp[:, 17 * h : 17 * h + 17],
                            AT[:, slot, :],
                            VA[:, kb, h, :],
                            start=(kb == 0 and slot == 0),
                            stop=(kb == qb),
                            skip_group_check=True,
                        )
                    if kb == qb:
                        # normalize: recip of denominator cols, broadcast mul
                        cp_ap = cp[:]
                        denom = bass.AP(
                            tensor=cp_ap.tensor,
                            offset=cp_ap[:, 16:17].offset,
                            ap=[cp_ap.ap[0], [17 * cp_ap.ap[1][0], HPG]],
                        )
                        R = work.tile([128, HPG], F32, tag="recip")
                        nc.vector.reciprocal(R[:], denom)
                        r_ap = R[:]
                        rb = bass.AP(
                            tensor=r_ap.tensor,
                            offset=r_ap.offset,
                            ap=[r_ap.ap[0], [r_ap.ap[1][0], HPG], [0, DH]],
                        )
                        nc.vector.tensor_tensor(
                            CTXD[qb][:],
                            cp_ap.rearrange("p (h c) -> p h c", c=17)[:, :, 0:DH],
                            rb,
                            mybir.AluOpType.mult,
                        )
                        if debug and qb == 1:
                            nc.sync.dma_start(dbg_cx[:], CTXD[1][:])
                        del ctx_ps[qb]

                for step in steps + [None] * CTX_DELAY:
                    if step is not None:
                        qb, kb = step
                        if kb == 0:
                            ctx_ps[qb] = ps_ctx.tile(
                                [128, HPG * 17], F32, tag="ctx", name=f"ctx{qb}"
                            )
                        diag = kb == qb
                        AT = work.tile([128, HPG, 128], BF16, tag="at")
                        for ti in range(2):
                            sc = ps_sc.tile([128, 8, 128], F32, tag="sc")
                            for slot in range(8):
                                h = head_of(ti, slot)
                                nc.tensor.matmul(
                                    sc[:, slot, :],
                                    head_slice(KT, KTs, h, kb),
                                    head_slice(QT, QTs, h, qb),
                                    start=(slot % 4 == 0),
                                    stop=(slot % 4 == 3),
                                    skip_group_check=True,
                                    tile_position=(96, 0)
                                    if 2 * ti + slot // 4 == 3
                                    else None,
                                )
                            if ti == 0:
                                # Act: native exp of tile0 (8 heads)
                                nc.scalar.activation(
                                    AT[:, 0:8, :],
                                    sc[:],
                                    mybir.ActivationFunctionType.Exp,
                                    scale=0.25,
                                )
                            else:
                                # Act: 1 slot; DVE: 7 slots via one-op
                                # Schraudolph exp into bf16 bits
                                nc.scalar.activation(
                                    AT[:, 8:9, :],
                                    sc[:, 0:1, :],
                                    mybir.ActivationFunctionType.Exp,
                                    scale=0.25,
                                )
                                nc.vector.tensor_scalar(
                                    out=AT[:, 9:16, :].bitcast(I16),
                                    in0=sc[:, 1:8, :],
                                    scalar1=SCH_C,
                                    scalar2=SCH_B,
                                    op0=mybir.AluOpType.mult,
                                    op1=mybir.AluOpType.add,
                                )
                        if diag:
                            # causal mask: zero exp'd entries with key j >
                            # query i (same pattern for every head slot)
                            nc.gpsimd.affine_select(
                                out=AT[:],
                                in_=AT[:],
                                compare_op=mybir.AluOpType.is_ge,
                                fill=0.0,
                                base=0,
                                pattern=[[0, HPG], [1, 128]],
                                channel_multiplier=-1,
                            )
                        if debug and step == (1, 1):
                            nc.sync.dma_start(dbg_at[:], AT[:])
                        pending.append((qb, kb, AT))
                    if len(pending) > (CTX_DELAY if step is not None else 0):
                        emit_ctx(*pending.pop(0))

            # ---- stage D: transpose + output projection ----
            CT = big.tile([128, 2, SEQ], BF16)
            with (
                tc.tile_pool(name="ps_tp", bufs=2, space="PSUM") as ps_tp,
                tc.tile_pool(name="ps_po", bufs=2, space="PSUM") as ps_po,
            ):
                for qb in range(NQB):
                    ctxf = CTXD[qb][:].rearrange("p h e -> p (h e)")
                    for c in range(2):
                        tp = ps_tp.tile([128, 128], BF16, tag="tp")
                        nc.tensor.transpose(
                            tp[:], ctxf[:, 128 * c : 128 * c + 128], IDENT[:]
                        )
                        nc.vector.tensor_copy(
                            CT[:, c, 128 * qb : 128 * qb + 128], tp[:]
                        )
                    po = ps_po.tile([128, EMB], F32, tag="po")
                    for dc in range(2):
                        for c in range(2):
                            nc.tensor.matmul(
                                po[:, 512 * dc : 512 * dc + 512],
                                CT[:, c, 128 * qb : 128 * qb + 128],
                                WO[:, c, 512 * dc : 512 * dc + 512],
                                start=(c == 0),
                                stop=(c == 1),
                            )
                    Y = work.tile([128, EMB], BF16, tag="y")
                    if qb % 2 == 0:
                        nc.scalar.copy(Y[:], po[:])
                    else:
                        nc.vector.tensor_copy(Y[:], po[:])
                    nc.sync.dma_start(y_d[128 * qb : 128 * qb + 128, :], Y[:])

    split_excess_waits(nc)
    return nc


_NC_CACHE = None


def _get_nc():
    global _NC_CACHE
    if _NC_CACHE is None:
        _NC_CACHE = build_nc()
    return _NC_CACHE


def kernel(x, Wq, Wk, Wv, Wo, bo):
    x = np.asarray(x, dtype=np.float32)
    Wq = np.asarray(Wq, dtype=np.float32)
    Wk = np.asarray(Wk, dtype=np.float32)
    Wv = np.asarray(Wv, dtype=np.float32)
    Wo = np.asarray(Wo, dtype=np.float32)
    bo = np.asarray(bo, dtype=np.float32)

    bf = ml_dtypes.bfloat16
    ident = np.eye(128, dtype=np.float32).astype(bf)

    def pack_w(W, cols):
        # [1024, 256] -> [128, 8, 256] with [p, kb, n] = W[128*kb+p, n]
        return np.ascontiguousarray(
            W[:, cols].reshape(8, 128, GCOLS).transpose(1, 0, 2)
        ).astype(bf)

    nc = _get_nc()
    in_maps = []
    for c in range(8):
        b, g = divmod(c, NG)
        cols = slice(GCOLS * g, GCOLS * g + GCOLS)
        xt = np.ascontiguousarray(
            x[b].T.reshape(8, 128, SEQ).transpose(1, 0, 2)
        ).astype(bf)
        wo = np.ascontiguousarray(
            Wo[cols, :].reshape(2, 128, EMB).transpose(1, 0, 2)
        ).astype(bf)
        in_maps.append(
            {
                "xt": xt,
                "wq": pack_w(Wq, cols),
                "wk": pack_w(Wk, cols),
                "wv": pack_w(Wv, cols),
                "wo": wo,
                "ident": ident,
            }
        )

    res = run_bass_kernel_spmd(nc, in_maps, core_ids=list(range(8)))
    out = np.zeros((BATCH, SEQ, EMB), dtype=np.float32)
    for c in range(8):
        b = c // NG
        out[b] += res.results[c]["y"].astype(np.float32)
    out += bo[None, None, :]
    return out


# revision 37
# speedup vs baseline: 1.4539x; 1.1674x over previous
"""Multi-head attention (axis-swapped variant) on 8 Trainium2 NeuronCores.

Reference semantics (EMB=1024, heads act on the d_head axis after the buggy
transpose): 64 effective heads of size 16, causal softmax scaled by
1/sqrt(16), projections Wq/Wk/Wv, output projection Wo + bo.

Sharding: core c = 4*b + g handles batch b and head-group g (16 heads =
256 contiguous projection columns). Each core returns a partial output
[1024, 1024] (bf16); the host sums the 4 group partials per batch and
adds bo.

Per-core structure:
  B) Q/K/V projections in bf16 (weights and x pre-cast on host).
  C) per (query-block qb, key-block kb<=qb) step:
     - 16 scores matmuls [128 keys, 128 queries] (one per head) into two
       2-bank PSUM tiles. Hardware constraint: all matmuls into one PSUM
       bank must share the stationary row-base (tile_position row), so
       heads are grouped per bank by their 32-aligned partition base:
       tile0 = heads {0,1,8,9 | 2,3,10,11}, tile1 = {4,5,12,13 |
       6,7,14,15}.
     - exp: Act engine takes tile0 + 1 slot (native Exp, scale 1/4), DVE
       takes 7 slots via one-op Schraudolph (int16(s*C+B) bitcast bf16,
       ~3% rel err on those heads).
     - diagonal steps: one GPSIMD affine_select zeroes the j>i half of
       the exp'd tile (causal mask), instead of masking scores.
     - ctx accumulated transposed: stationary = attn tile [128k, 128q],
       moving = V_aug [128k, 17] (16 v cols + ones col). Output
       [128 q, 16 heads * 17] accumulates over kb in one PSUM bank; the
       ones column yields the softmax denominator per (q, head). ctx is
       emitted 2 steps late so PE never waits on exp/affsel.
     - after the diagonal step: reciprocal of the 16 denominator cols,
       broadcast-multiply normalizes -> CTXD[qb] [128 q, 256] bf16.
  D) per qb: PE-transpose CTXD into [256 he, 128 q] (bf16 identity),
     out-projection po[q, d] = sum_he CT[he, q] * Wo[he, d], store y.
"""

import numpy as np
import ml_dtypes

import concourse.bass as bass
import concourse.mybir as mybir
import concourse.tile as tile
from concourse.bass_utils import run_bass_kernel_spmd

F32 = mybir.dt.float32
BF16 = mybir.dt.bfloat16
I16 = mybir.dt.int16

EMB = 1024
SEQ = 1024
BATCH = 2
NG = 4            # head groups (cores per batch)
HPG = 16          # heads per group/core
DH = 16           # per-head feature size
GCOLS = HPG * DH  # 256 projection columns per core

NQB = 8           # query blocks of 128
CTX_DELAY = 3     # steps between scores emission and ctx consumption

SCH_C = 128.0 * float(np.log2(np.e)) * 0.25
SCH_B = 127.0 * 128.0 - 7.0


def head_of(ti, slot):
    """Head stored in scores tile `ti` slot `slot` (4 slots per bank; each
    bank holds only heads whose 16-row q/k slice starts at one 32-aligned
    partition base)."""
    bi = 2 * ti + slot // 4
    i = slot % 4
    return [2 * bi, 2 * bi + 1, 8 + 2 * bi, 8 + 2 * bi + 1][i]


def split_excess_waits(nc, cap=1):
    """This container's walrus rejects instructions carrying more than a few
    semaphore waits (and bass's own model says one). Relocate excess waits
    onto preceding same-engine EventSemaphore instructions."""

    def fix_block(bb, dummy):
        insts = bb.instructions
        i = 0
        while i < len(insts):
            inst = insts[i]
            si = inst.sync_info
            waits = list(si.on_wait) if si is not None and si.on_wait else []
            if len(waits) > cap:
                eng = nc.engines[inst.engine]
                excess, keep = waits[:-cap], waits[-cap:]
                si.on_wait = keep
                pos = i
                for j in range(0, len(excess), cap):
                    chunk = excess[j : j + cap]
                    ev = eng.wait_ge(dummy, 1)
                    cur_list = nc.cur_bb.bb.instructions
                    assert cur_list[-1] is ev.ins
                    cur_list.pop()
                    ev.ins.sync_info.on_wait = chunk
                    insts.insert(pos, ev.ins)
                    pos += 1
                    i += 1
            i += 1

    with nc.semaphore("waitfix_dummy") as dummy:
        for f in nc.m.functions:
            for bb in f.blocks:
                fix_block(bb, dummy)


def build_nc(debug=False):
    nc = bass.Bass()
    xt_d = nc.declare_dram_parameter("xt", [128, 8, SEQ], BF16, isOutput=False)
    wq_d = nc.declare_dram_parameter("wq", [128, 8, GCOLS], BF16, isOutput=False)
    wk_d = nc.declare_dram_parameter("wk", [128, 8, GCOLS], BF16, isOutput=False)
    wv_d = nc.declare_dram_parameter("wv", [128, 8, GCOLS], BF16, isOutput=False)
    wo_d = nc.declare_dram_parameter("wo", [128, 2, EMB], BF16, isOutput=False)
    id_d = nc.declare_dram_parameter("ident", [128, 128], BF16, isOutput=False)
    y_d = nc.declare_dram_parameter("y", [SEQ, EMB], BF16, isOutput=True)
    if debug:
        dbg_at = nc.declare_dram_parameter("dbg_at", [128, HPG, 128], BF16, isOutput=True)
        dbg_cx = nc.declare_dram_parameter("dbg_cx", [128, HPG, DH], BF16, isOutput=True)

    with tile.TileContext(nc) as tc:
        with (
            tc.tile_pool(name="big", bufs=1) as big,
            tc.tile_pool(name="work", bufs=6) as work,
        ):
            # ---- stage A: loads (wq first so projections start early) ----
            XT = big.tile([128, 8, SEQ], BF16)
            WQ = big.tile([128, 8, GCOLS], BF16)
            WK = big.tile([128, 8, GCOLS], BF16)
            WV = big.tile([128, 8, GCOLS], BF16)
            WO = big.tile([128, 2, EMB], BF16)
            IDENT = big.tile([128, 128], BF16)
            nc.sync.dma_start(WQ[:], wq_d[:])
            for kb in range(8):
                nc.sync.dma_start(XT[:, kb, :], xt_d[:, kb, :])
            nc.sync.dma_start(WK[:], wk_d[:])
            nc.sync.dma_start(WV[:], wv_d[:])
            nc.sync.dma_start(IDENT[:], id_d[:])
            nc.sync.dma_start(WO[:], wo_d[:])

            # v-augmented moving tiles: per (kb, head) a [128, 17] block =
            # [v(16 cols) | ones]
            VA = big.tile([128, 8, HPG, 17], BF16)
            nc.gpsimd.memset(VA[:, :, :, 16:17], 1.0)

            # ---- stage B: projections (bf16) ----
            QT = big.tile([128, 2, SEQ], BF16)
            KT = big.tile([128, 2, SEQ], BF16)
            with tc.tile_pool(name="ps_proj", bufs=1, space="PSUM") as ps_proj:
                # kb-major: all 4 quarters of a projection accumulate together
                # so each arriving XT chunk feeds 4 matmuls (PE keeps pace
                # with the DMA feed)
                for W, T in ((WQ, QT), (WK, KT)):
                    pqs = [
                        ps_proj.tile([128, 512], F32, tag=f"pq{i}", name=f"pq{i}")
                        for i in range(4)
                    ]
                    for kb in range(8):
                        for t in range(2):
                            for ic in range(2):
                                nc.tensor.matmul(
                                    pqs[2 * t + ic][:],
                                    W[:, kb, 128 * t : 128 * t + 128],
                                    XT[:, kb, 512 * ic : 512 * ic + 512],
                                    start=(kb == 0),
                                    stop=(kb == 7),
                                )
                    for t in range(2):
                        for ic in range(2):
                            nc.scalar.copy(
                                T[:, t, 512 * ic : 512 * ic + 512],
                                pqs[2 * t + ic][:],
                            )
                # v natural: out[m, col] = sum_d xT[d, m] * W[d, col]
                for half in range(2):
                    pvs = [
                        ps_proj.tile([128, GCOLS], F32, tag=f"pv{i}", name=f"pv{i}")
                        for i in range(4)
                    ]
                    for kb in range(8):
                        for i in range(4):
                            mt = 4 * half + i
                            nc.tensor.matmul(
                                pvs[i][:],
                                XT[:, kb, 128 * mt : 128 * mt + 128],
                                WV[:, kb, :],
                                start=(kb == 0),
                                stop=(kb == 7),
                            )
                    for i in range(4):
                        nc.vector.tensor_copy(
                            VA[:, 4 * half + i, :, 0:16],
                            pvs[i][:].rearrange("p (h e) -> p h e", e=DH),
                        )

            # shifted copies: odd heads moved down 16 partitions so every
            # head's 16 rows start at a 32-aligned partition
            QTs = big.tile([128, 2, SEQ], BF16)
            KTs = big.tile([128, 2, SEQ], BF16)
            for srcT, dstT in ((QT, QTs), (KT, KTs)):
                for j in range(4):
                    nc.sync.dma_start(
                        dstT[32 * j : 32 * j + 16, :, :],
                        srcT[32 * j + 16 : 32 * j + 32, :, :],
                    )

            def head_slice(T, Ts, h, blk):
                t, hl = divmod(h, 8)
                src = T if hl % 2 == 0 else Ts
                base = 16 * (hl - hl % 2)
                return src[base : base + 16, t, 128 * blk : 128 * blk + 128]

            # ---- stage C: attention ----
            CTXD = [
                big.tile([128, HPG, DH], BF16, tag=f"ctxd{qb}", name=f"ctxd{qb}")
                for qb in range(NQB)
            ]
            steps = [(qb, kb) for qb in range(NQB) for kb in range(qb + 1)]
            with (
                tc.tile_pool(name="ps_sc", bufs=3, space="PSUM") as ps_sc,
                tc.tile_pool(name="ps_ctx", bufs=2, space="PSUM") as ps_ctx,
            ):
                ctx_ps = {}
                pending = []

                def emit_ctx(qb, kb, AT):
                    # start=True resets the whole PSUM bank (pending-zero
                    # covers the bank), so only the first matmul into the
                    # bank carries it; first writes to untouched regions
                    # still initialize via the per-byte pending-zero flags.
                    cp = ctx_ps[qb]
                    for slot in range(HPG):
                        h = head_of(slot // 8, slot % 8)
                        nc.tensor.matmul(
                            cp[:, 17 * h : 17 * h + 17],
                            AT[:, slot, :],
                            VA[:, kb, h, :],
                            start=(kb == 0 and slot == 0),
                            stop=(kb == qb),
                            skip_group_check=True,
                        )
                    if kb == qb:
                        # normalize: recip of denominator cols, broadcast mul
                        cp_ap = cp[:]
                        denom = bass.AP(
                            tensor=cp_ap.tensor,
                            offset=cp_ap[:, 16:17].offset,
                            ap=[cp_ap.ap[0], [17 * cp_ap.ap[1][0], HPG]],
                        )
                        R = work.tile([128, HPG], F32, tag="recip")
                        nc.vector.reciprocal(R[:], denom)
                        r_ap = R[:]
                        rb = bass.AP(
                            tensor=r_ap.tensor,
                            offset=r_ap.offset,
                            ap=[r_ap.ap[0], [r_ap.ap[1][0], HPG], [0, DH]],
                        )
                        nc.vector.tensor_tensor(
                            CTXD[qb][:],
                            cp_ap.rearrange("p (h c) -> p h c", c=17)[:, :, 0:DH],
                            rb,
                            mybir.AluOpType.mult,
                        )
                        if debug and qb == 1:
                            nc.sync.dma_start(dbg_cx[:], CTXD[1][:])
                        del ctx_ps[qb]

                for step in steps + [None] * CTX_DELAY:
                    if step is not None:
                        qb, kb = step
                        if kb == 0:
                            ctx_ps[qb] = ps_ctx.tile(
                                [128, HPG * 17], F32, tag="ctx", name=f"ctx{qb}"
                            )
                        diag = kb == qb
                        AT = work.tile([128, HPG, 128], BF16, tag="at")
                        for ti in range(2):
                            sc = ps_sc.tile([128, 8, 128], F32, tag="sc")
                            for slot in range(8):
                                h = head_of(ti, slot)
                                nc.tensor.matmul(
                                    sc[:, slot, :],
                                    head_slice(KT, KTs, h, kb),
                                    head_slice(QT, QTs, h, qb),
                                    start=(slot % 4 == 0),
                                    stop=(slot % 4 == 3),
                                    skip_group_check=True,
                                    tile_position=(96, 0)
                                    if 2 * ti + slot // 4 == 3
                                    else None,
                                )
                            if ti == 0:
                                # Act: native exp of tile0 (8 heads)
                                nc.scalar.activation(
                                    AT[:, 0:8, :],
                                    sc[:],
                                    mybir.ActivationFunctionType.Exp,
                                    scale=0.25,
                                )
                            else:
                                # DVE: 8 heads via one-op Schraudolph exp
                                # into bf16 bits
                                nc.vector.tensor_scalar(
                                    out=AT[:, 8:16, :].bitcast(I16),
                                    in0=sc[:],
                                    scalar1=SCH_C,
                                    scalar2=SCH_B,
                                    op0=mybir.AluOpType.mult,
                                    op1=mybir.AluOpType.add,
                                )
                        if diag:
                            # causal mask: zero exp'd entries with key j >
                            # query i (same pattern for every head slot)
                            nc.gpsimd.affine_select(
                                out=AT[:],
                                in_=AT[:],
                                compare_op=mybir.AluOpType.is_ge,
                                fill=0.0,
                                base=0,
                                pattern=[[0, HPG], [1, 128]],
                                channel_multiplier=-1,
                            )
                        if debug and step == (1, 1):
                            nc.sync.dma_start(dbg_at[:], AT[:])
                        pending.append((qb, kb, AT))
                    if len(pending) > (CTX_DELAY if step is not None else 0):
                        emit_ctx(*pending.pop(0))

            # ---- stage D: transpose + output projection ----
            CT = big.tile([128, 2, SEQ], BF16)
            with (
                tc.tile_pool(name="ps_tp", bufs=1, space="PSUM") as ps_tp,
                tc.tile_pool(name="ps_po", bufs=2, space="PSUM") as ps_po,
            ):
                # all 16 transposes first (2 one-bank psum tiles of 8 slices),
                # copies drain to CT while the out-projections stream on PE
                tps = []
                for c in range(2):
                    tp = ps_tp.tile([128, 8, 128], BF16, tag=f"tp{c}", name=f"tp{c}")
                    tps.append(tp)
                for qb in range(NQB):
                    ctxf = CTXD[qb][:].rearrange("p h e -> p (h e)")
                    for c in range(2):
                        nc.tensor.matmul(
                            tps[c][:, qb, :],
                            ctxf[:, 128 * c : 128 * c + 128],
                            IDENT[:],
                            is_transpose=True,
                            start=(qb == 0),
                            stop=(qb == 7),
                            skip_group_check=True,
                        )
                for qb in range(NQB):
                    for c in range(2):
                        if qb % 2 == 0:
                            nc.scalar.copy(
                                CT[:, c, 128 * qb : 128 * qb + 128],
                                tps[c][:, qb, :],
                            )
                        else:
                            nc.vector.tensor_copy(
                                CT[:, c, 128 * qb : 128 * qb + 128],
                                tps[c][:, qb, :],
                            )
                for qb in range(NQB):
                    po = ps_po.tile([128, EMB], F32, tag="po")
                    for dc in range(2):
                        for c in range(2):
                            nc.tensor.matmul(
                                po[:, 512 * dc : 512 * dc + 512],
                                CT[:, c, 128 * qb : 128 * qb + 128],
                                WO[:, c, 512 * dc : 512 * dc + 512],
                                start=(c == 0),
                                stop=(c == 1),
                            )
                    Y = work.tile([128, EMB], BF16, tag="y")
                    if qb % 2 == 0:
                        nc.scalar.copy(Y[:], po[:])
                    else:
                        nc.vector.tensor_copy(Y[:], po[:])
                    nc.sync.dma_start(y_d[128 * qb : 128 * qb + 128, :], Y[:])

    split_excess_waits(nc)
    return nc


_NC_CACHE = None


def _get_nc():
    global _NC_CACHE
    if _NC_CACHE is None:
        _NC_CACHE = build_nc()
    return _NC_CACHE


def kernel(x, Wq, Wk, Wv, Wo, bo):
    x = np.asarray(x, dtype=np.float32)
    Wq = np.asarray(Wq, dtype=np.float32)
    Wk = np.asarray(Wk, dtype=np.float32)
    Wv = np.asarray(Wv, dtype=np.float32)
    Wo = np.asarray(Wo, dtype=np.float32)
    bo = np.asarray(bo, dtype=np.float32)

    bf = ml_dtypes.bfloat16
    ident = np.eye(128, dtype=np.float32).astype(bf)

    def pack_w(W, cols):
        # [1024, 256] -> [128, 8, 256] with [p, kb, n] = W[128*kb+p, n]
        return np.ascontiguousarray(
            W[:, cols].reshape(8, 128, GCOLS).transpose(1, 0, 2)
        ).astype(bf)

    nc = _get_nc()
    in_maps = []
    for c in range(8):
        b, g = divmod(c, NG)
        cols = slice(GCOLS * g, GCOLS * g + GCOLS)
        xt = np.ascontiguousarray(
            x[b].T.reshape(8, 128, SEQ).transpose(1, 0, 2)
        ).astype(bf)
        wo = np.ascontiguousarray(
            Wo[cols, :].reshape(2, 128, EMB).transpose(1, 0, 2)
        ).astype(bf)
        in_maps.append(
            {
                "xt": xt,
                "wq": pack_w(Wq, cols),
                "wk": pack_w(Wk, cols),
                "wv": pack_w(Wv, cols),
                "wo": wo,
                "ident": ident,
            }
        )

    res = run_bass_kernel_spmd(nc, in_maps, core_ids=list(range(8)))
    out = np.zeros((BATCH, SEQ, EMB), dtype=np.float32)
    for c in range(8):
        b = c // NG
        out[b] += res.results[c]["y"].astype(np.float32)
    out += bo[None, None, :]
    return out


# revision 39
# speedup vs baseline: 1.5867x; 1.0913x over previous
"""Multi-head attention (axis-swapped variant) on 8 Trainium2 NeuronCores.

Reference semantics (EMB=1024, heads act on the d_head axis after the buggy
transpose): 64 effective heads of size 16, causal softmax scaled by
1/sqrt(16), projections Wq/Wk/Wv, output projection Wo + bo.

Sharding: core c = 4*b + g handles batch b and head-group g (16 heads =
256 contiguous projection columns). Each core returns a partial output
[1024, 1024] (bf16); the host sums the 4 group partials per batch and
adds bo.

Per-core structure:
  B) Q/K/V projections in bf16 (weights and x pre-cast on host).
  C) per (query-block qb, key-block kb<=qb) step:
     - 16 scores matmuls [128 keys, 128 queries] (one per head) into two
       2-bank PSUM tiles. Hardware constraint: all matmuls into one PSUM
       bank must share the stationary row-base (tile_position row), so
       heads are grouped per bank by their 32-aligned partition base:
       tile0 = heads {0,1,8,9 | 2,3,10,11}, tile1 = {4,5,12,13 |
       6,7,14,15}.
     - exp: Act engine takes tile0 + 1 slot (native Exp, scale 1/4), DVE
       takes 7 slots via one-op Schraudolph (int16(s*C+B) bitcast bf16,
       ~3% rel err on those heads).
     - diagonal steps: one GPSIMD affine_select zeroes the j>i half of
       the exp'd tile (causal mask), instead of masking scores.
     - ctx accumulated transposed: stationary = attn tile [128k, 128q],
       moving = V_aug [128k, 17] (16 v cols + ones col). Output
       [128 q, 16 heads * 17] accumulates over kb in one PSUM bank; the
       ones column yields the softmax denominator per (q, head). ctx is
       emitted 2 steps late so PE never waits on exp/affsel.
     - after the diagonal step: reciprocal of the 16 denominator cols,
       broadcast-multiply normalizes -> CTXD[qb] [128 q, 256] bf16.
  D) per qb: PE-transpose CTXD into [256 he, 128 q] (bf16 identity),
     out-projection po[q, d] = sum_he CT[he, q] * Wo[he, d], store y.
"""

import numpy as np
import ml_dtypes

import concourse.bass as bass
import concourse.mybir as mybir
import concourse.tile as tile
from concourse.bass_utils import run_bass_kernel_spmd

F32 = mybir.dt.float32
BF16 = mybir.dt.bfloat16
I16 = mybir.dt.int16

EMB = 1024
SEQ = 1024
BATCH = 2
NG = 4            # head groups (cores per batch)
HPG = 16          # heads per group/core
DH = 16           # per-head feature size
GCOLS = HPG * DH  # 256 projection columns per core

NQB = 8           # query blocks of 128
CTX_DELAY = 3     # steps between scores emission and ctx consumption

SCH_C = 128.0 * float(np.log2(np.e)) * 0.25
SCH_B = 127.0 * 128.0 - 7.0


def head_of(ti, slot):
    """Head stored in scores tile `ti` slot `slot` (4 slots per bank; each
    bank holds only heads whose 16-row q/k slice starts at one 32-aligned
    partition base)."""
    bi = 2 * ti + slot // 4
    i = slot % 4
    return [2 * bi, 2 * bi + 1, 8 + 2 * bi, 8 + 2 * bi + 1][i]


def split_excess_waits(nc, cap=1):
    """This container's walrus rejects instructions carrying more than a few
    semaphore waits (and bass's own model says one). Relocate excess waits
    onto preceding same-engine EventSemaphore instructions."""

    def fix_block(bb, dummy):
        insts = bb.instructions
        i = 0
        while i < len(insts):
            inst = insts[i]
            si = inst.sync_info
            waits = list(si.on_wait) if si is not None and si.on_wait else []
            if len(waits) > cap:
                eng = nc.engines[inst.engine]
                excess, keep = waits[:-cap], waits[-cap:]
                si.on_wait = keep
                pos = i
                for j in range(0, len(excess), cap):
                    chunk = excess[j : j + cap]
                    ev = eng.wait_ge(dummy, 1)
                    cur_list = nc.cur_bb.bb.instructions
                    assert cur_list[-1] is ev.ins
                    cur_list.pop()
                    ev.ins.sync_info.on_wait = chunk
                    insts.insert(pos, ev.ins)
                    pos += 1
                    i += 1
            i += 1

    with nc.semaphore("waitfix_dummy") as dummy:
        for f in nc.m.functions:
            for bb in f.blocks:
                fix_block(bb, dummy)


def build_nc(debug=False):
    nc = bass.Bass()
    xt_d = nc.declare_dram_parameter("xt", [128, 8, SEQ], BF16, isOutput=False)
    wq_d = nc.declare_dram_parameter("wq", [128, 8, GCOLS], BF16, isOutput=False)
    wk_d = nc.declare_dram_parameter("wk", [128, 8, GCOLS], BF16, isOutput=False)
    wv_d = nc.declare_dram_parameter("wv", [128, 8, GCOLS], BF16, isOutput=False)
    wo_d = nc.declare_dram_parameter("wo", [128, 2, EMB], BF16, isOutput=False)
    id_d = nc.declare_dram_parameter("ident", [128, 128], BF16, isOutput=False)
    y_d = nc.declare_dram_parameter("y", [SEQ, EMB], BF16, isOutput=True)
    if debug:
        dbg_at = nc.declare_dram_parameter("dbg_at", [128, HPG, 128], BF16, isOutput=True)
        dbg_cx = nc.declare_dram_parameter("dbg_cx", [128, HPG, DH], BF16, isOutput=True)

    with tile.TileContext(nc) as tc:
        with (
            tc.tile_pool(name="big", bufs=1) as big,
            tc.tile_pool(name="work", bufs=6) as work,
        ):
            # ---- stage A: loads (wq first so projections start early) ----
            XT = big.tile([128, 8, SEQ], BF16)
            WQ = big.tile([128, 8, GCOLS], BF16)
            WK = big.tile([128, 8, GCOLS], BF16)
            WV = big.tile([128, 8, GCOLS], BF16)
            WO = big.tile([128, 2, EMB], BF16)
            IDENT = big.tile([128, 128], BF16)
            nc.sync.dma_start(WQ[:, 0:2, :], wq_d[:, 0:2, :])
            nc.sync.dma_start(XT[:, 0, :], xt_d[:, 0, :])
            nc.sync.dma_start(WQ[:, 2:8, :], wq_d[:, 2:8, :])
            for kb in range(1, 8):
                nc.sync.dma_start(XT[:, kb, :], xt_d[:, kb, :])
            nc.sync.dma_start(WK[:], wk_d[:])
            nc.sync.dma_start(WV[:], wv_d[:])
            nc.sync.dma_start(IDENT[:], id_d[:])
            nc.sync.dma_start(WO[:], wo_d[:])

            # v-augmented moving tiles: per (kb, head) a [128, 17] block =
            # [v(16 cols) | ones]
            VA = big.tile([128, 8, HPG, 17], BF16)
            nc.gpsimd.memset(VA[:, :, :, 16:17], 1.0)

            # ---- stage B: projections (bf16) ----
            QT = big.tile([128, 2, SEQ], BF16)
            KT = big.tile([128, 2, SEQ], BF16)
            WUP = big.tile([128, 128], BF16)
            nc.gpsimd.memset(WUP[:], 0.0)
            with tc.tile_pool(name="ps_proj", bufs=1, space="PSUM") as ps_proj:
                # warm the PE p-state while the first DMAs land
                pw = ps_proj.tile([128, 512], F32, tag="pq0", name="pwarm")
                for i in range(30):
                    nc.tensor.matmul(
                        pw[:, 0:128],
                        WUP[:],
                        WUP[:],
                        start=True,
                        stop=True,
                        skip_group_check=True,
                    )
                # kb-major: all 4 quarters of a projection accumulate together
                # so each arriving XT chunk feeds 4 matmuls (PE keeps pace
                # with the DMA feed)
                for W, T in ((WQ, QT), (WK, KT)):
                    pqs = [
                        ps_proj.tile([128, 512], F32, tag=f"pq{i}", name=f"pq{i}")
                        for i in range(4)
                    ]
                    for kb in range(8):
                        for t in range(2):
                            for ic in range(2):
                                nc.tensor.matmul(
                                    pqs[2 * t + ic][:],
                                    W[:, kb, 128 * t : 128 * t + 128],
                                    XT[:, kb, 512 * ic : 512 * ic + 512],
                                    start=(kb == 0),
                                    stop=(kb == 7),
                                )
                    for t in range(2):
                        for ic in range(2):
                            nc.scalar.copy(
                                T[:, t, 512 * ic : 512 * ic + 512],
                                pqs[2 * t + ic][:],
                            )
                # v natural: out[m, col] = sum_d xT[d, m] * W[d, col]
                for half in range(2):
                    pvs = [
                        ps_proj.tile([128, GCOLS], F32, tag=f"pv{i}", name=f"pv{i}")
                        for i in range(4)
                    ]
                    for kb in range(8):
                        for i in range(4):
                            mt = 4 * half + i
                            nc.tensor.matmul(
                                pvs[i][:],
                                XT[:, kb, 128 * mt : 128 * mt + 128],
                                WV[:, kb, :],
                                start=(kb == 0),
                                stop=(kb == 7),
                            )
                    for i in range(4):
                        nc.vector.tensor_copy(
                            VA[:, 4 * half + i, :, 0:16],
                            pvs[i][:].rearrange("p (h e) -> p h e", e=DH),
                        )

            # shifted copies: odd heads moved down 16 partitions so every
            # head's 16 rows start at a 32-aligned partition
            QTs = big.tile([128, 2, SEQ], BF16)
            KTs = big.tile([128, 2, SEQ], BF16)
            for srcT, dstT in ((QT, QTs), (KT, KTs)):
                for j in range(4):
                    nc.sync.dma_start(
                        dstT[32 * j : 32 * j + 16, :, :],
                        srcT[32 * j + 16 : 32 * j + 32, :, :],
                    )

            def head_slice(T, Ts, h, blk):
                t, hl = divmod(h, 8)
                src = T if hl % 2 == 0 else Ts
                base = 16 * (hl - hl % 2)
                return src[base : base + 16, t, 128 * blk : 128 * blk + 128]

            # ---- stage C: attention ----
            CTXD = [
                big.tile([128, HPG, DH], BF16, tag=f"ctxd{qb}", name=f"ctxd{qb}")
                for qb in range(NQB)
            ]
            steps = [(qb, kb) for qb in range(NQB) for kb in range(qb + 1)]
            CT = big.tile([128, 2, SEQ], BF16)
            with (
                tc.tile_pool(name="ps_sc", bufs=3, space="PSUM") as ps_sc,
                tc.tile_pool(name="ps_ctx", bufs=1, space="PSUM") as ps_ctx,
                tc.tile_pool(name="ps_tp", bufs=1, space="PSUM") as ps_tp,
            ):
                ctx_ps = {}
                pending = []

                def emit_ctx(qb, kb, AT):
                    # start=True resets the whole PSUM bank (pending-zero
                    # covers the bank), so only the first matmul into the
                    # bank carries it; first writes to untouched regions
                    # still initialize via the per-byte pending-zero flags.
                    cp = ctx_ps[qb]
                    for slot in range(HPG):
                        h = head_of(slot // 8, slot % 8)
                        nc.tensor.matmul(
                            cp[:, 17 * h : 17 * h + 17],
                            AT[:, slot, :],
                            VA[:, kb, h, :],
                            start=(kb == 0 and slot == 0),
                            stop=(kb == qb),
                            skip_group_check=True,
                        )
                    if kb == qb:
                        # normalize: recip of denominator cols, broadcast mul
                        cp_ap = cp[:]
                        denom = bass.AP(
                            tensor=cp_ap.tensor,
                            offset=cp_ap[:, 16:17].offset,
                            ap=[cp_ap.ap[0], [17 * cp_ap.ap[1][0], HPG]],
                        )
                        R = work.tile([128, HPG], F32, tag="recip")
                        nc.vector.reciprocal(R[:], denom)
                        r_ap = R[:]
                        rb = bass.AP(
                            tensor=r_ap.tensor,
                            offset=r_ap.offset,
                            ap=[r_ap.ap[0], [r_ap.ap[1][0], HPG], [0, DH]],
                        )
                        nc.vector.tensor_tensor(
                            CTXD[qb][:],
                            cp_ap.rearrange("p (h c) -> p h c", c=17)[:, :, 0:DH],
                            rb,
                            mybir.AluOpType.mult,
                        )
                        if debug and qb == 1:
                            nc.sync.dma_start(dbg_cx[:], CTXD[1][:])
                        del ctx_ps[qb]
                        ctxf = CTXD[qb][:].rearrange("p h e -> p (h e)")
                        tp = ps_tp.tile([128, 2, 128], BF16, tag="tp")
                        for c in range(2):
                            nc.tensor.transpose(
                                tp[:, c, :], ctxf[:, 128 * c : 128 * c + 128],
                                IDENT[:],
                            )
                            nc.scalar.copy(
                                CT[:, c, 128 * qb : 128 * qb + 128], tp[:, c, :]
                            )

                for step in steps + [None] * CTX_DELAY:
                    if step is not None:
                        qb, kb = step
                        if kb == 0:
                            ctx_ps[qb] = ps_ctx.tile(
                                [128, HPG * 17], F32, tag="ctx", name=f"ctx{qb}"
                            )
                        diag = kb == qb
                        AT = work.tile([128, HPG, 128], BF16, tag="at")
                        for ti in range(2):
                            sc = ps_sc.tile([128, 8, 128], F32, tag="sc")
                            for slot in range(8):
                                h = head_of(ti, slot)
                                nc.tensor.matmul(
                                    sc[:, slot, :],
                                    head_slice(KT, KTs, h, kb),
                                    head_slice(QT, QTs, h, qb),
                                    start=(slot % 4 == 0),
                                    stop=(slot % 4 == 3),
                                    skip_group_check=True,
                                    tile_position=(96, 0)
                                    if 2 * ti + slot // 4 == 3
                                    else None,
                                )
                            if ti == 0:
                                # Act: native exp of tile0 (8 heads)
                                nc.scalar.activation(
                                    AT[:, 0:8, :],
                                    sc[:],
                                    mybir.ActivationFunctionType.Exp,
                                    scale=0.25,
                                )
                            else:
                                # DVE: 8 heads via one-op Schraudolph exp
                                # into bf16 bits
                                nc.vector.tensor_scalar(
                                    out=AT[:, 8:16, :].bitcast(I16),
                                    in0=sc[:],
                                    scalar1=SCH_C,
                                    scalar2=SCH_B,
                                    op0=mybir.AluOpType.mult,
                                    op1=mybir.AluOpType.add,
                                )
                        if diag:
                            # causal mask: zero exp'd entries with key j >
                            # query i (same pattern for every head slot)
                            nc.gpsimd.affine_select(
                                out=AT[:],
                                in_=AT[:],
                                compare_op=mybir.AluOpType.is_ge,
                                fill=0.0,
                                base=0,
                                pattern=[[0, HPG], [1, 128]],
                                channel_multiplier=-1,
                            )
                        if debug and step == (1, 1):
                            nc.sync.dma_start(dbg_at[:], AT[:])
                        pending.append((qb, kb, AT))
                    if len(pending) > (CTX_DELAY if step is not None else 0):
                        emit_ctx(*pending.pop(0))

            # ---- stage D: output projection (transposes done in stage C) ----
            with (
                tc.tile_pool(name="ps_po", bufs=2, space="PSUM") as ps_po,
            ):
                for qb in range(NQB):
                    po = ps_po.tile([128, EMB], F32, tag="po")
                    for dc in range(2):
                        for c in range(2):
                            nc.tensor.matmul(
                                po[:, 512 * dc : 512 * dc + 512],
                                CT[:, c, 128 * qb : 128 * qb + 128],
                                WO[:, c, 512 * dc : 512 * dc + 512],
                                start=(c == 0),
                                stop=(c == 1),
                            )
                    Y = work.tile([128, EMB], BF16, tag="y")
                    if qb % 2 == 0:
                        nc.scalar.copy(Y[:], po[:])
                    else:
                        nc.vector.tensor_copy(Y[:], po[:])
                    nc.sync.dma_start(y_d[128 * qb : 128 * qb + 128, :], Y[:])

    split_excess_waits(nc)
    return nc


_NC_CACHE = None


def _get_nc():
    global _NC_CACHE
    if _NC_CACHE is None:
        _NC_CACHE = build_nc()
    return _NC_CACHE


def kernel(x, Wq, Wk, Wv, Wo, bo):
    x = np.asarray(x, dtype=np.float32)
    Wq = np.asarray(Wq, dtype=np.float32)
    Wk = np.asarray(Wk, dtype=np.float32)
    Wv = np.asarray(Wv, dtype=np.float32)
    Wo = np.asarray(Wo, dtype=np.float32)
    bo = np.asarray(bo, dtype=np.float32)

    bf = ml_dtypes.bfloat16
    ident = np.eye(128, dtype=np.float32).astype(bf)

    def pack_w(W, cols):
        # [1024, 256] -> [128, 8, 256] with [p, kb, n] = W[128*kb+p, n]
        return np.ascontiguousarray(
            W[:, cols].reshape(8, 128, GCOLS).transpose(1, 0, 2)
        ).astype(bf)

    nc = _get_nc()
    in_maps = []
    for c in range(8):
        b, g = divmod(c, NG)
        cols = slice(GCOLS * g, GCOLS * g + GCOLS)
        xt = np.ascontiguousarray(
            x[b].T.reshape(8, 128, SEQ).transpose(1, 0, 2)
        ).astype(bf)
        wo = np.ascontiguousarray(
            Wo[cols, :].reshape(2, 128, EMB).transpose(1, 0, 2)
        ).astype(bf)
        in_maps.append(
            {
                "xt": xt,
                "wq": pack_w(Wq, cols),
                "wk": pack_w(Wk, cols),
                "wv": pack_w(Wv, cols),
                "wo": wo,
                "ident": ident,
            }
        )

    res = run_bass_kernel_spmd(nc, in_maps, core_ids=list(range(8)))
    out = np.zeros((BATCH, SEQ, EMB), dtype=np.float32)
    for c in range(8):
        b = c // NG
        out[b] += res.results[c]["y"].astype(np.float32)
    out += bo[None, None, :]
    return out
